# revision 23
# baseline (speedup 1.0000x reference)
"""Causal self-attention (B=2, N=4096, C=768, 12 heads, d=64) on 8 trn2 cores.

Sharding: core (b, g) = batch b, head-group g (3 heads). Tensor-parallel on
heads: each core computes qkv projection for its 3 heads, causal flash
attention, and a partial output projection; host sums the 4 partials per batch
and adds b_out.

Device layout notes:
 - All matmuls bf16 (fp32 PSUM accumulation).
 - Qt/Kt produced directly in [d, seq] layout by using W as the stationary
   matmul operand against host-pretransposed xT.
 - Scores St in [k, q] layout; probs = exp(St/8) with NO max subtraction
   (scores are bounded ~|2|), causal mask applied as a 0/1 bf16 multiply on
   diagonal blocks only.
 - PV: stationary [V_h | ones | 0] (66 cols) -> O^T rows 0-63, softmax
   denominator in row 64 for free.
 - Weight-group layout [q0|q1][k0|k1][q2|q2][k2|k2] lets QK^T run pairs of
   K=64 matmuls concurrently on disjoint PE row groups.
"""

import numpy as np
import ml_dtypes

import concourse.bass as bass
import concourse.mybir as mybir
import concourse.tile as tile
from concourse import bass_utils
from concourse.vector_clock import ScopedClock

P = 128
D = 64
C = 768
HL = 3          # heads per core
QT = 512        # q tile width
VW = 66 * HL    # v sbuf row width: [v_h(64) | ones | zero] x 3
N_CORES = 8
BF = mybir.dt.bfloat16
F32 = mybir.dt.float32
BF_NP = ml_dtypes.bfloat16


class PatchedTileContext(tile.TileContext):
    """This toolchain's walrus rejects more than ONE sync-wait on any
    instruction ("Too many sync wait commands"). Tile's wait assignment
    freely attaches several. Legalize: for every instruction with k>1
    waits, insert k-1 same-engine NOPs before it, one wait each."""

    def _split_sync_waits(self):
        nc = self.nc
        for bb in nc.m.functions[0].blocks:
            insts = bb.instructions
            out = []
            changed = False
            for inst in insts:
                si = inst.sync_info
                waits = list(si.on_wait or []) if si is not None else []
                if len(waits) > 1:
                    changed = True
                    for w in waits[:-1]:
                        nop = mybir.InstNoOp(
                            name=f"I-wsplit{nc.next_id()}", text_hint="wsplit")
                        nop.engine = inst.engine
                        nop.sync_info = mybir.SyncInfo(on_wait=[w], on_update=[])
                        nc.register_instruction(nop)
                        out.append(nop)
                    si.on_wait = waits[-1:]
                out.append(inst)
            if changed:
                bb.instructions = out

    def _drain_and_barrier(self, tick_clock, wait_clock):
        drain_inst = self.nc.sync.drain()
        wait_clock.add_sem_waits(
            drain_inst.ins, ScopedClock({None: tick_clock.global_clock})
        )
        si = drain_inst.ins.sync_info
        waits = list(si.on_wait or []) if si is not None else []
        if len(waits) > 1:
            si.on_wait = waits[:1]
            for w in waits[1:]:
                extra = self.nc.sync.drain()
                esi = extra.ins.sync_info
                if esi is None:
                    extra.ins.sync_info = mybir.SyncInfo(on_wait=[w], on_update=[])
                else:
                    esi.on_wait = [w]

        self.nc.all_engine_barrier()
        assert self.sems is not None
        popped = self.nc._tile_sem_poison_stack.pop()
        assert popped is self._sem_poison
        # clear_and_free_semaphores would emit EVENT_SEMAPHORE_RANGE_CLEAR
        # (an InstISA), which this walrus rejects ("ISA wrong length") — and
        # per-sem sem_clear lowers to the same opcode. Skip the clears: this
        # is the only TileContext in the NEFF and NRT re-initializes
        # semaphores per execution (verified empirically by repeated runs).
        self.nc.all_engine_barrier()
        self._split_sync_waits()


def build_nc(n_seq=4096):
    CC = C // P                  # 6 contraction chunks
    NQ = n_seq // QT             # q tiles
    nc = bass.Bass("TRN2", target_bir_lowering=False, debug=False,
                   num_devices=N_CORES)

    xT = nc.dram_tensor("xT", [C, n_seq], BF, kind="ExternalInput").ap()
    wqk = nc.dram_tensor("wqk", [C, 4 * P], BF, kind="ExternalInput").ap()
    bqk = nc.dram_tensor("bqk", [P, 4], F32, kind="ExternalInput").ap()
    wv = nc.dram_tensor("wv", [C, VW], BF, kind="ExternalInput").ap()
    bv = nc.dram_tensor("bv", [P, VW], F32, kind="ExternalInput").ap()
    wo = nc.dram_tensor("wo", [D, HL, C], BF, kind="ExternalInput").ap()
    mask = nc.dram_tensor("mask", [P, 4, QT], BF, kind="ExternalInput").ap()
    out = nc.dram_tensor("out", [n_seq, C], F32, kind="ExternalOutput").ap()

    Exp = mybir.ActivationFunctionType.Exp

    from contextlib import ExitStack
    with PatchedTileContext(nc) as tc, ExitStack() as ctx:
        consts = ctx.enter_context(tc.tile_pool(name="consts", bufs=1))
        # weights / constants
        wqk_sb = consts.tile([P, CC, 4 * P], BF, name="wqk_sb")
        nc.sync.dma_start(wqk_sb[:], wqk.rearrange("(o p) m -> p o m", p=P))
        bqk_sb = consts.tile([P, 4], F32, name="bqk_sb")
        nc.sync.dma_start(bqk_sb[:], bqk[:])
        wv_sb = consts.tile([P, CC, VW], BF, name="wv_sb")
        nc.sync.dma_start(wv_sb[:], wv.rearrange("(o p) m -> p o m", p=P))
        bv_sb = consts.tile([P, VW], F32, name="bv_sb")
        nc.sync.dma_start(bv_sb[:], bv[:])
        wo_sb = consts.tile([D, HL, C], BF, name="wo_sb")
        nc.sync.dma_start(wo_sb[:], wo[:])
        mask_sb = consts.tile([P, 4, QT], BF, name="mask_sb")
        nc.sync.dma_start(mask_sb[:], mask[:])
        xt_sb = []
        for c in range(CC):
            t = consts.tile([P, n_seq], BF, name=f"xt{c}")
            nc.sync.dma_start(t[:], xT[c * P:(c + 1) * P, :])
            xt_sb.append(t)
        # persistent intermediates
        qkt = [[consts.tile([P, QT], BF, name=f"qkt{g}_{s}") for s in range(NQ)]
               for g in range(4)]
        v_s = [consts.tile([P, VW], BF, name=f"v{s}") for s in range(4 * NQ)]
        ot = [consts.tile([D, n_seq], BF, name=f"ot{h}") for h in range(HL)]

        ps_gen = ctx.enter_context(tc.tile_pool(name="ps_gen", bufs=2, space="PSUM"))
        ps_st = ctx.enter_context(tc.tile_pool(name="ps_st", bufs=2, space="PSUM"))
        ps_pv = ctx.enter_context(tc.tile_pool(name="ps_pv", bufs=2, space="PSUM"))
        sb_pt = ctx.enter_context(tc.tile_pool(name="sb_pt", bufs=4))
        sb_nrm = ctx.enter_context(tc.tile_pool(name="sb_nrm", bufs=2))
        sb_out = ctx.enter_context(tc.tile_pool(name="sb_out", bufs=2))
        dr_nrm = ctx.enter_context(tc.tile_pool(name="dr_nrm", bufs=2, space="DRAM"))

        for s in range(NQ):
            qsl = slice(QT * s, QT * (s + 1))
            # ---- qkv projection for this seq tile ----
            for g in range(4):
                ps = ps_gen.tile([P, QT], F32, tag="gen", name=f"psqk{g}_{s}")
                for c in range(CC):
                    nc.tensor.matmul(
                        ps[:], wqk_sb[:, c, P * g:P * (g + 1)],
                        xt_sb[c][:, qsl],
                        start=(c == 0), stop=(c == CC - 1))
                nc.vector.tensor_add(qkt[g][s][:], ps[:],
                                     bqk_sb[:, g:g + 1].to_broadcast((P, QT)))
            for sc in range(4 * s, 4 * s + 4):
                ps = ps_gen.tile([P, QT], F32, tag="gen", name=f"psv{sc}")
                for c in range(CC):
                    nc.tensor.matmul(
                        ps[:, :VW], xt_sb[c][:, P * sc:P * (sc + 1)],
                        wv_sb[:, c, :],
                        start=(c == 0), stop=(c == CC - 1))
                nc.vector.tensor_add(v_s[sc][:], ps[:, :VW], bv_sb[:])

            # ---- attention for q-tile j = s ----
            j = s
            nkc = 4 * (j + 1)          # causal k chunks
            # heads 0,1: one St tile per k chunk, slots = heads (row-paired)
            pv01 = [ps_pv.tile([D + 2, QT], F32, tag="pv", name=f"pv{j}_{h}")
                    for h in range(2)]
            for kc in range(nkc):
                si, co = kc // 4, P * (kc % 4)
                stp = ps_st.tile([P, 2, QT], F32, tag="st", name=f"st{j}_{kc}")
                for h in range(2):
                    r = slice(D * h, D * (h + 1))
                    nc.tensor.matmul(stp[:, h, :],
                                     qkt[1][si][r, co:co + P],
                                     qkt[0][j][r, :],
                                     start=True, stop=True)
                pt = sb_pt.tile([P, 2, QT], BF, tag="pt", name=f"pt{j}_{kc}")
                nc.scalar.activation(pt[:], stp[:], Exp, scale=0.125)
                if kc >= 4 * j:
                    m = mask_sb[:, kc - 4 * j, :]
                    for h in range(2):
                        nc.vector.tensor_mul(pt[:, h, :], pt[:, h, :], m)
                for h in range(2):
                    nc.tensor.matmul(pv01[h][:],
                                     v_s[kc][:, 66 * h:66 * h + 66],
                                     pt[:, h, :],
                                     start=(kc == 0), stop=(kc == nkc - 1))
            # head 2: slots = consecutive k chunks (self row-paired)
            pv2 = ps_pv.tile([D + 2, QT], F32, tag="pv", name=f"pv{j}_2")
            for gk in range(nkc // 2):
                st2 = ps_st.tile([P, 2, QT], F32, tag="st", name=f"st2_{j}_{gk}")
                for cr in range(2):
                    kc = 2 * gk + cr
                    si, co = kc // 4, P * (kc % 4)
                    r = slice(D * (kc % 2), D * (kc % 2) + D)
                    nc.tensor.matmul(st2[:, cr, :],
                                     qkt[3][si][r, co:co + P],
                                     qkt[2][j][r, :],
                                     start=True, stop=True)
                pt = sb_pt.tile([P, 2, QT], BF, tag="pt", name=f"pt2_{j}_{gk}")
                nc.scalar.activation(pt[:], st2[:], Exp, scale=0.125)
                if gk >= 2 * j:
                    for cr in range(2):
                        m = mask_sb[:, 2 * (gk - 2 * j) + cr, :]
                        nc.vector.tensor_mul(pt[:, cr, :], pt[:, cr, :], m)
                for cr in range(2):
                    kc = 2 * gk + cr
                    nc.tensor.matmul(pv2[:],
                                     v_s[kc][:, 66 * 2:66 * 2 + 66],
                                     pt[:, cr, :],
                                     start=(kc == 0), stop=(kc == nkc - 1))
            # normalize O^T by the denominators (psum row 64): batched per-j
            # chain — den rows -> DRAM -> [128, 3*QT/128] so the exact 6-cpe
            # DVE reciprocal uses all lanes, then bounce back replicated.
            # Bounce DMAs ride the idle Pool engine's SWDGE so the dependent
            # chain never blocks SP's bulk DMA queue.
            for h, pvp in enumerate(pv01 + [pv2]):
                den = sb_nrm.tile([P, QT], F32, tag="den", name=f"den{j}_{h}")
                nc.vector.tensor_copy(den[D:D + 1, :], pvp[D:D + 1, :])
                scr = dr_nrm.tile([QT], F32, tag="scr", name=f"scr{j}_{h}")
                nc.gpsimd.dma_start(scr[None, :], den[D:D + 1, :])
                dfold = sb_nrm.tile([P, QT // P], F32, tag="dfold",
                                    name=f"dfold{j}_{h}")
                nc.gpsimd.dma_start(dfold[:],
                                    scr.rearrange("(p f) -> p f", p=P))
                rfold = sb_nrm.tile([P, QT // P], F32, tag="rfold",
                                    name=f"rfold{j}_{h}")
                nc.vector.reciprocal(rfold[:], dfold[:])
                scr2 = dr_nrm.tile([QT], F32, tag="scr2", name=f"scr2{j}_{h}")
                nc.gpsimd.dma_start(scr2.rearrange("(p f) -> p f", p=P),
                                    rfold[:])
                rep = sb_nrm.tile([D, QT], F32, tag="rep", name=f"rep{j}_{h}")
                nc.gpsimd.dma_start(rep[:],
                                    scr2[None, :].to_broadcast((D, QT)))
                nc.vector.tensor_mul(ot[h][:, qsl], pvp[0:D, :], rep[:])

            # ---- output projection, one iteration behind (keeps the norm
            # chain latency off the critical path) ----
            for jp in ([j - 1] if j > 0 else []) + ([j] if j == NQ - 1 else []):
                for qc in range(4 * jp, 4 * jp + 4):
                    osb = sb_out.tile([P, C], F32, tag="osb", name=f"osb{qc}")
                    for nh in range(2):
                        pj = ps_gen.tile([P, QT], F32, tag="gen",
                                         name=f"pj{qc}_{nh}")
                        nsl = slice(384 * nh, 384 * (nh + 1))
                        for h in range(HL):
                            nc.tensor.matmul(pj[:, :384],
                                             ot[h][:, P * qc:P * (qc + 1)],
                                             wo_sb[:, h, nsl],
                                             start=(h == 0), stop=(h == HL - 1))
                        nc.vector.tensor_copy(osb[:, nsl], pj[:, :384])
                    nc.sync.dma_start(out[P * qc:P * (qc + 1), :], osb[:])

    return nc


def make_mask():
    p = np.arange(P)[:, None, None]
    c = np.arange(4)[None, :, None]
    qf = np.arange(QT)[None, None, :]
    return (qf >= P * c + p).astype(BF_NP)


def prep_core_inputs(x, W_attn, b_attn, W_out, b, g, mask):
    """Host-side shard prep for core (batch b, head group g)."""
    habs = [HL * g + h for h in range(HL)]
    wq = [W_attn[:, D * h:D * (h + 1)] for h in habs]
    wk = [W_attn[:, C + D * h:C + D * (h + 1)] for h in habs]
    wv_ = [W_attn[:, 2 * C + D * h:2 * C + D * (h + 1)] for h in habs]
    bq = [b_attn[D * h:D * (h + 1)] for h in habs]
    bk = [b_attn[C + D * h:C + D * (h + 1)] for h in habs]
    bvv = [b_attn[2 * C + D * h:2 * C + D * (h + 1)] for h in habs]

    wqk = np.concatenate(
        [wq[0], wq[1], wk[0], wk[1], wq[2], wq[2], wk[2], wk[2]], axis=1)
    bqk = np.stack([
        np.concatenate([bq[0], bq[1]]),
        np.concatenate([bk[0], bk[1]]),
        np.concatenate([bq[2], bq[2]]),
        np.concatenate([bk[2], bk[2]]),
    ], axis=1).astype(np.float32)

    wv_ext = np.zeros((C, VW), dtype=np.float32)
    bv_ext = np.zeros(VW, dtype=np.float32)
    for h in range(HL):
        wv_ext[:, 66 * h:66 * h + D] = wv_[h]
        bv_ext[66 * h:66 * h + D] = bvv[h]
        bv_ext[66 * h + D] = 1.0
    bv_tile = np.ascontiguousarray(
        np.broadcast_to(bv_ext, (P, VW))).astype(np.float32)

    wo = np.ascontiguousarray(
        W_out[192 * g:192 * (g + 1), :].reshape(HL, D, C).transpose(1, 0, 2))

    return {
        "xT": np.ascontiguousarray(x[b].T).astype(BF_NP),
        "wqk": wqk.astype(BF_NP),
        "bqk": bqk,
        "wv": wv_ext.astype(BF_NP),
        "bv": bv_tile,
        "wo": wo.astype(BF_NP),
        "mask": mask,
    }


_NC_CACHE = {}


def kernel(x, W_attn, b_attn, W_out, b_out):
    x = np.asarray(x, dtype=np.float32)
    W_attn = np.asarray(W_attn, dtype=np.float32)
    b_attn = np.asarray(b_attn, dtype=np.float32)
    W_out = np.asarray(W_out, dtype=np.float32)
    b_out = np.asarray(b_out, dtype=np.float32)
    B, n_seq, _ = x.shape

    if n_seq not in _NC_CACHE:
        _NC_CACHE[n_seq] = build_nc(n_seq)
    nc = _NC_CACHE[n_seq]

    mask = make_mask()
    in_maps = [prep_core_inputs(x, W_attn, b_attn, W_out, b, g, mask)
               for b in range(B) for g in range(4)]
    res = bass_utils.run_bass_kernel_spmd(
        nc, in_maps, core_ids=list(range(N_CORES)))
    parts = [r["out"] for r in res.results]
    out = np.empty((B, n_seq, C), dtype=np.float32)
    for b in range(B):
        out[b] = parts[4 * b] + parts[4 * b + 1] + parts[4 * b + 2] \
            + parts[4 * b + 3] + b_out
    return out


# revision 25
# speedup vs baseline: 11693.6551x; 11693.6551x over previous
"""Causal self-attention (B=2, N=4096, C=768, 12 heads, d=64) on 8 trn2 cores.

Sharding: core (b, g) = batch b, head-group g (3 heads). Tensor-parallel on
heads: each core computes qkv projection for its 3 heads, causal flash
attention, and a partial output projection; host sums the 4 partials per batch
and adds b_out.

Device layout notes:
 - All matmuls bf16 (fp32 PSUM accumulation).
 - Qt/Kt produced directly in [d, seq] layout by using W as the stationary
   matmul operand against host-pretransposed xT.
 - Scores St in [k, q] layout; probs = exp(St/8) with NO max subtraction
   (scores are bounded ~|2|), causal mask applied as a 0/1 bf16 multiply on
   diagonal blocks only.
 - PV: stationary [V_h | ones | 0] (66 cols) -> O^T rows 0-63, softmax
   denominator in row 64 for free.
 - Weight-group layout [q0|q1][k0|k1][q2|q2][k2|k2] lets QK^T run pairs of
   K=64 matmuls concurrently on disjoint PE row groups.
"""

import numpy as np
import ml_dtypes

import concourse.bass as bass
import concourse.mybir as mybir
import concourse.tile as tile
from concourse import bass_utils
from concourse.vector_clock import ScopedClock

P = 128
D = 64
C = 768
HL = 3          # heads per core
QT = 512        # q tile width
VW = 66 * HL    # v sbuf row width: [v_h(64) | ones | zero] x 3
N_CORES = 8
BF = mybir.dt.bfloat16
F32 = mybir.dt.float32
BF_NP = ml_dtypes.bfloat16


class PatchedTileContext(tile.TileContext):
    """This toolchain's walrus rejects more than ONE sync-wait on any
    instruction ("Too many sync wait commands"). Tile's wait assignment
    freely attaches several. Legalize: for every instruction with k>1
    waits, insert k-1 same-engine NOPs before it, one wait each."""

    def _split_sync_waits(self):
        nc = self.nc
        for bb in nc.m.functions[0].blocks:
            insts = bb.instructions
            out = []
            changed = False
            for inst in insts:
                si = inst.sync_info
                waits = list(si.on_wait or []) if si is not None else []
                if len(waits) > 1:
                    changed = True
                    for w in waits[:-1]:
                        nop = mybir.InstNoOp(
                            name=f"I-wsplit{nc.next_id()}", text_hint="wsplit")
                        nop.engine = inst.engine
                        nop.sync_info = mybir.SyncInfo(on_wait=[w], on_update=[])
                        nc.register_instruction(nop)
                        out.append(nop)
                    si.on_wait = waits[-1:]
                out.append(inst)
            if changed:
                bb.instructions = out

    def _drain_and_barrier(self, tick_clock, wait_clock):
        drain_inst = self.nc.sync.drain()
        wait_clock.add_sem_waits(
            drain_inst.ins, ScopedClock({None: tick_clock.global_clock})
        )
        si = drain_inst.ins.sync_info
        waits = list(si.on_wait or []) if si is not None else []
        if len(waits) > 1:
            si.on_wait = waits[:1]
            for w in waits[1:]:
                extra = self.nc.sync.drain()
                esi = extra.ins.sync_info
                if esi is None:
                    extra.ins.sync_info = mybir.SyncInfo(on_wait=[w], on_update=[])
                else:
                    esi.on_wait = [w]

        self.nc.all_engine_barrier()
        assert self.sems is not None
        popped = self.nc._tile_sem_poison_stack.pop()
        assert popped is self._sem_poison
        # clear_and_free_semaphores would emit EVENT_SEMAPHORE_RANGE_CLEAR
        # (an InstISA), which this walrus rejects ("ISA wrong length") — and
        # per-sem sem_clear lowers to the same opcode. Skip the clears: this
        # is the only TileContext in the NEFF and NRT re-initializes
        # semaphores per execution (verified empirically by repeated runs).
        self.nc.all_engine_barrier()
        self._split_sync_waits()


def build_nc(n_seq=4096):
    CC = C // P                  # 6 contraction chunks
    NQ = n_seq // QT             # q tiles
    nc = bass.Bass("TRN2", target_bir_lowering=False, debug=False,
                   num_devices=N_CORES)

    xT = nc.dram_tensor("xT", [C, n_seq], BF, kind="ExternalInput").ap()
    wqk = nc.dram_tensor("wqk", [C, 4 * P], BF, kind="ExternalInput").ap()
    bqk = nc.dram_tensor("bqk", [P, 4], F32, kind="ExternalInput").ap()
    wv = nc.dram_tensor("wv", [C, VW], BF, kind="ExternalInput").ap()
    bv = nc.dram_tensor("bv", [P, VW], F32, kind="ExternalInput").ap()
    wo = nc.dram_tensor("wo", [D, HL, C], BF, kind="ExternalInput").ap()
    mask = nc.dram_tensor("mask", [P, 4, QT], BF, kind="ExternalInput").ap()
    out = nc.dram_tensor("out", [n_seq, C], F32, kind="ExternalOutput").ap()

    Exp = mybir.ActivationFunctionType.Exp

    from contextlib import ExitStack
    with PatchedTileContext(nc) as tc, ExitStack() as ctx:
        consts = ctx.enter_context(tc.tile_pool(name="consts", bufs=1))
        # weights / constants
        wqk_sb = consts.tile([P, CC, 4 * P], BF, name="wqk_sb")
        nc.sync.dma_start(wqk_sb[:], wqk.rearrange("(o p) m -> p o m", p=P))
        bqk_sb = consts.tile([P, 4], F32, name="bqk_sb")
        nc.sync.dma_start(bqk_sb[:], bqk[:])
        wv_sb = consts.tile([P, CC, VW], BF, name="wv_sb")
        nc.sync.dma_start(wv_sb[:], wv.rearrange("(o p) m -> p o m", p=P))
        bv_sb = consts.tile([P, VW], F32, name="bv_sb")
        nc.sync.dma_start(bv_sb[:], bv[:])
        wo_sb = consts.tile([D, HL, C], BF, name="wo_sb")
        nc.sync.dma_start(wo_sb[:], wo[:])
        mask_sb = consts.tile([P, 4, QT], BF, name="mask_sb")
        nc.sync.dma_start(mask_sb[:], mask[:])
        xt_sb = []
        for c in range(CC):
            t = consts.tile([P, n_seq], BF, name=f"xt{c}")
            nc.sync.dma_start(t[:], xT[c * P:(c + 1) * P, :])
            xt_sb.append(t)
        # persistent intermediates
        qkt = [[consts.tile([P, QT], BF, name=f"qkt{g}_{s}") for s in range(NQ)]
               for g in range(4)]
        v_s = [consts.tile([P, VW], BF, name=f"v{s}") for s in range(4 * NQ)]
        ot = [consts.tile([D, n_seq], BF, name=f"ot{h}") for h in range(HL)]

        ps_gen = ctx.enter_context(tc.tile_pool(name="ps_gen", bufs=2, space="PSUM"))
        ps_st = ctx.enter_context(tc.tile_pool(name="ps_st", bufs=2, space="PSUM"))
        ps_pv = ctx.enter_context(tc.tile_pool(name="ps_pv", bufs=2, space="PSUM"))
        sb_pt = ctx.enter_context(tc.tile_pool(name="sb_pt", bufs=4))
        sb_nrm = ctx.enter_context(tc.tile_pool(name="sb_nrm", bufs=2))
        sb_out = ctx.enter_context(tc.tile_pool(name="sb_out", bufs=2))
        dr_nrm = ctx.enter_context(tc.tile_pool(name="dr_nrm", bufs=2, space="DRAM"))

        for s in range(NQ):
            qsl = slice(QT * s, QT * (s + 1))
            # ---- qkv projection for this seq tile ----
            for g in range(4):
                ps = ps_gen.tile([P, QT], F32, tag="gen", name=f"psqk{g}_{s}")
                for c in range(CC):
                    nc.tensor.matmul(
                        ps[:], wqk_sb[:, c, P * g:P * (g + 1)],
                        xt_sb[c][:, qsl],
                        start=(c == 0), stop=(c == CC - 1))
                nc.vector.tensor_add(qkt[g][s][:], ps[:],
                                     bqk_sb[:, g:g + 1].to_broadcast((P, QT)))
            for sc in range(4 * s, 4 * s + 4):
                ps = ps_gen.tile([P, QT], F32, tag="gen", name=f"psv{sc}")
                for c in range(CC):
                    nc.tensor.matmul(
                        ps[:, :VW], xt_sb[c][:, P * sc:P * (sc + 1)],
                        wv_sb[:, c, :],
                        start=(c == 0), stop=(c == CC - 1))
                nc.vector.tensor_add(v_s[sc][:], ps[:, :VW], bv_sb[:])

            # ---- attention for q-tile j = s ----
            j = s
            nkc = 4 * (j + 1)          # causal k chunks
            # heads 0,1: one St tile per k chunk, slots = heads (row-paired)
            pv01 = [ps_pv.tile([D + 2, QT], F32, tag="pv", name=f"pv{j}_{h}")
                    for h in range(2)]
            for kc in range(nkc):
                si, co = kc // 4, P * (kc % 4)
                # causal: columns below off are fully masked for this chunk —
                # skip them in QK^T and exp (stale psum there is never read),
                # zero them in pt, and mask only the triangular boundary.
                off = P * (kc - 4 * j) if kc >= 4 * j else 0
                stp = ps_st.tile([P, 2, QT], F32, tag="st", name=f"st{j}_{kc}")
                for h in range(2):
                    r = slice(D * h, D * (h + 1))
                    nc.tensor.matmul(stp[:, h, off:],
                                     qkt[1][si][r, co:co + P],
                                     qkt[0][j][r, off:],
                                     start=True, stop=True)
                pt = sb_pt.tile([P, 2, QT], BF, tag="pt", name=f"pt{j}_{kc}")
                if off:
                    nc.vector.memset(pt[:, :, :off], 0.0)
                nc.scalar.activation(pt[:, :, off:], stp[:, :, off:],
                                     Exp, scale=0.125)
                if kc >= 4 * j:
                    m = mask_sb[:, kc - 4 * j, off:off + P]
                    for h in range(2):
                        nc.vector.tensor_mul(pt[:, h, off:off + P],
                                             pt[:, h, off:off + P], m)
                for h in range(2):
                    nc.tensor.matmul(pv01[h][:],
                                     v_s[kc][:, 66 * h:66 * h + 66],
                                     pt[:, h, :],
                                     start=(kc == 0), stop=(kc == nkc - 1))
            # head 2: slots = consecutive k chunks (self row-paired)
            pv2 = ps_pv.tile([D + 2, QT], F32, tag="pv", name=f"pv{j}_2")
            for gk in range(nkc // 2):
                st2 = ps_st.tile([P, 2, QT], F32, tag="st", name=f"st2_{j}_{gk}")
                for cr in range(2):
                    kc = 2 * gk + cr
                    si, co = kc // 4, P * (kc % 4)
                    r = slice(D * (kc % 2), D * (kc % 2) + D)
                    nc.tensor.matmul(st2[:, cr, :],
                                     qkt[3][si][r, co:co + P],
                                     qkt[2][j][r, :],
                                     start=True, stop=True)
                pt = sb_pt.tile([P, 2, QT], BF, tag="pt", name=f"pt2_{j}_{gk}")
                nc.scalar.activation(pt[:], st2[:], Exp, scale=0.125)
                if gk >= 2 * j:
                    for cr in range(2):
                        m = mask_sb[:, 2 * (gk - 2 * j) + cr, :]
                        nc.vector.tensor_mul(pt[:, cr, :], pt[:, cr, :], m)
                for cr in range(2):
                    kc = 2 * gk + cr
                    nc.tensor.matmul(pv2[:],
                                     v_s[kc][:, 66 * 2:66 * 2 + 66],
                                     pt[:, cr, :],
                                     start=(kc == 0), stop=(kc == nkc - 1))
            # normalize O^T by the denominators (psum row 64): batched per-j
            # chain — den rows -> DRAM -> [128, 3*QT/128] so the exact 6-cpe
            # DVE reciprocal uses all lanes, then bounce back replicated.
            # Bounce DMAs ride the idle Pool engine's SWDGE so the dependent
            # chain never blocks SP's bulk DMA queue.
            for h, pvp in enumerate(pv01 + [pv2]):
                # stage O^T and den out of PSUM immediately: frees the pv
                # psum slot after two quick DVE copies, so the bounce chain
                # below never gates the next q-tile's PV matmuls.
                osg = sb_nrm.tile([D, QT], F32, tag="osg", name=f"osg{j}_{h}")
                nc.vector.tensor_copy(osg[:], pvp[0:D, :])
                den = sb_nrm.tile([P, QT], F32, tag="den", name=f"den{j}_{h}")
                nc.vector.tensor_copy(den[D:D + 1, :], pvp[D:D + 1, :])
                scr = dr_nrm.tile([QT], F32, tag="scr", name=f"scr{j}_{h}")
                nc.gpsimd.dma_start(scr[None, :], den[D:D + 1, :])
                dfold = sb_nrm.tile([P, QT // P], F32, tag="dfold",
                                    name=f"dfold{j}_{h}")
                nc.gpsimd.dma_start(dfold[:],
                                    scr.rearrange("(p f) -> p f", p=P))
                rfold = sb_nrm.tile([P, QT // P], F32, tag="rfold",
                                    name=f"rfold{j}_{h}")
                nc.vector.reciprocal(rfold[:], dfold[:])
                scr2 = dr_nrm.tile([QT], F32, tag="scr2", name=f"scr2{j}_{h}")
                nc.gpsimd.dma_start(scr2.rearrange("(p f) -> p f", p=P),
                                    rfold[:])
                rep = sb_nrm.tile([D, QT], F32, tag="rep", name=f"rep{j}_{h}")
                nc.gpsimd.dma_start(rep[:],
                                    scr2[None, :].to_broadcast((D, QT)))
                nc.vector.tensor_mul(ot[h][:, qsl], osg[:], rep[:])

            # ---- output projection, one iteration behind (keeps the norm
            # chain latency off the critical path) ----
            for jp in ([j - 1] if j > 0 else []) + ([j] if j == NQ - 1 else []):
                for qc in range(4 * jp, 4 * jp + 4):
                    osb = sb_out.tile([P, C], F32, tag="osb", name=f"osb{qc}")
                    for nh in range(2):
                        pj = ps_gen.tile([P, QT], F32, tag="gen",
                                         name=f"pj{qc}_{nh}")
                        nsl = slice(384 * nh, 384 * (nh + 1))
                        for h in range(HL):
                            nc.tensor.matmul(pj[:, :384],
                                             ot[h][:, P * qc:P * (qc + 1)],
                                             wo_sb[:, h, nsl],
                                             start=(h == 0), stop=(h == HL - 1))
                        nc.vector.tensor_copy(osb[:, nsl], pj[:, :384])
                    nc.sync.dma_start(out[P * qc:P * (qc + 1), :], osb[:])

    return nc


def make_mask():
    p = np.arange(P)[:, None, None]
    c = np.arange(4)[None, :, None]
    qf = np.arange(QT)[None, None, :]
    return (qf >= P * c + p).astype(BF_NP)


def prep_core_inputs(x, W_attn, b_attn, W_out, b, g, mask):
    """Host-side shard prep for core (batch b, head group g)."""
    habs = [HL * g + h for h in range(HL)]
    wq = [W_attn[:, D * h:D * (h + 1)] for h in habs]
    wk = [W_attn[:, C + D * h:C + D * (h + 1)] for h in habs]
    wv_ = [W_attn[:, 2 * C + D * h:2 * C + D * (h + 1)] for h in habs]
    bq = [b_attn[D * h:D * (h + 1)] for h in habs]
    bk = [b_attn[C + D * h:C + D * (h + 1)] for h in habs]
    bvv = [b_attn[2 * C + D * h:2 * C + D * (h + 1)] for h in habs]

    wqk = np.concatenate(
        [wq[0], wq[1], wk[0], wk[1], wq[2], wq[2], wk[2], wk[2]], axis=1)
    bqk = np.stack([
        np.concatenate([bq[0], bq[1]]),
        np.concatenate([bk[0], bk[1]]),
        np.concatenate([bq[2], bq[2]]),
        np.concatenate([bk[2], bk[2]]),
    ], axis=1).astype(np.float32)

    wv_ext = np.zeros((C, VW), dtype=np.float32)
    bv_ext = np.zeros(VW, dtype=np.float32)
    for h in range(HL):
        wv_ext[:, 66 * h:66 * h + D] = wv_[h]
        bv_ext[66 * h:66 * h + D] = bvv[h]
        bv_ext[66 * h + D] = 1.0
    bv_tile = np.ascontiguousarray(
        np.broadcast_to(bv_ext, (P, VW))).astype(np.float32)

    wo = np.ascontiguousarray(
        W_out[192 * g:192 * (g + 1), :].reshape(HL, D, C).transpose(1, 0, 2))

    return {
        "xT": np.ascontiguousarray(x[b].T).astype(BF_NP),
        "wqk": wqk.astype(BF_NP),
        "bqk": bqk,
        "wv": wv_ext.astype(BF_NP),
        "bv": bv_tile,
        "wo": wo.astype(BF_NP),
        "mask": mask,
    }


_NC_CACHE = {}


def kernel(x, W_attn, b_attn, W_out, b_out):
    x = np.asarray(x, dtype=np.float32)
    W_attn = np.asarray(W_attn, dtype=np.float32)
    b_attn = np.asarray(b_attn, dtype=np.float32)
    W_out = np.asarray(W_out, dtype=np.float32)
    b_out = np.asarray(b_out, dtype=np.float32)
    B, n_seq, _ = x.shape

    if n_seq not in _NC_CACHE:
        _NC_CACHE[n_seq] = build_nc(n_seq)
    nc = _NC_CACHE[n_seq]

    mask = make_mask()
    in_maps = [prep_core_inputs(x, W_attn, b_attn, W_out, b, g, mask)
               for b in range(B) for g in range(4)]
    res = bass_utils.run_bass_kernel_spmd(
        nc, in_maps, core_ids=list(range(N_CORES)))
    parts = [r["out"] for r in res.results]
    out = np.empty((B, n_seq, C), dtype=np.float32)
    for b in range(B):
        out[b] = parts[4 * b] + parts[4 * b + 1] + parts[4 * b + 2] \
            + parts[4 * b + 3] + b_out
    return out
